# revision 3
# baseline (speedup 1.0000x reference)
"""Trainium2 Bass kernel for nn_Decoder (dense MLP heads + 2-iter GCN refine).

Sharding: data-parallel over batch B across 8 NeuronCores; each core owns
batch rows [c*1024, (c+1)*1024) == graph nodes [c*65536, (c+1)*65536).
Edges are partitioned by dst ownership; the per-iteration message table
m = (x_masked @ Wg) is AllGathered so gathers stay in local HBM.

Self-contained: hardcodes all shapes; no sibling imports.
"""

import math

import numpy as np

import concourse.bacc as bacc
import concourse.bass as bass
import concourse.mybir as mybir
from concourse import tile
from concourse.bass_utils import run_bass_kernel_spmd
from concourse.masks import make_identity

B, NN, BBX, CLS, LAT = 8192, 64, 6, 16, 128
IN = LAT + NN + CLS  # 208
NT = B * NN  # 524288
NCORES = 8
BC = B // NCORES  # 1024 batch rows per core
NLOC = NT // NCORES  # 65536 nodes per core
NPC = NLOC // 128  # 512 node columns per partition (node n -> p=n//512, c=n%512)
F32 = mybir.dt.float32
I32 = mybir.dt.int32
AF = mybir.ActivationFunctionType
OP = mybir.AluOpType
AX = mybir.AxisListType

IDX_BLK = 1024  # chunk columns preloaded per SBUF block
PROFILE = False
LAST_EXEC_NS = None


def _prep_edges(E):
    """Sort/partition edges by dst core and dst tile; pad to 128-edge chunks.

    Node-major mapping inside a core: node local id d -> tile t = d % NPC,
    slot (partition of the output tile) = d // NPC.
    Returns (K, blocks, idxh, nmh, rlh) where K[t] = #chunks of tile t
    (common across cores), and the [8][128, CH] host arrays.
    """
    src = E[0].astype(np.int64)
    dst = E[1].astype(np.int64)
    deg = np.bincount(dst, minlength=NT).astype(np.float64) + 1.0
    dinv = 1.0 / np.sqrt(deg)
    loops = np.arange(NT, dtype=np.int64)
    src_all = np.concatenate([src, loops])
    dst_all = np.concatenate([dst, loops])
    norm_all = (dinv[src_all] * dinv[dst_all]).astype(np.float32)

    percore = []
    cnts = np.zeros((NCORES, NPC), np.int64)
    for c in range(NCORES):
        m = (dst_all // NLOC) == c
        s = src_all[m]
        d = dst_all[m] - c * NLOC
        w = norm_all[m]
        t = d % NPC
        order = np.argsort(t, kind="stable")
        s, d, w, t = s[order], d[order], w[order], t[order]
        cnts[c] = np.bincount(t, minlength=NPC)
        percore.append((s, d, w))
    K = np.maximum(1, np.ceil(cnts / 128.0)).astype(np.int64).max(axis=0)

    # pack tiles into IDX_BLK-column blocks without crossing block edges
    blocks = []  # list of list of tile ids
    cur, used = [], 0
    for t in range(NPC):
        k = int(K[t])
        if used + k > IDX_BLK:
            blocks.append(cur)
            cur, used = [], 0
        cur.append(t)
        used += k
    if cur:
        blocks.append(cur)

    CH = len(blocks) * IDX_BLK
    idxh = np.zeros((NCORES, CH, 128), np.int32)
    nmh = np.zeros((NCORES, CH, 128), np.float32)
    rlh = np.zeros((NCORES, CH, 128), np.float32)
    for c in range(NCORES):
        s, d, w = percore[c]
        estart = np.concatenate([[0], np.cumsum(cnts[c])])
        ch0 = 0
        for blk in blocks:
            off = 0
            for t in blk:
                k = int(K[t])
                e0, e1 = int(estart[t]), int(estart[t + 1])
                n = e1 - e0
                bi = np.zeros(k * 128, np.int32)
                bn = np.zeros(k * 128, np.float32)
                br = np.zeros(k * 128, np.float32)
                bi[:n] = s[e0:e1]
                bn[:n] = w[e0:e1]
                br[:n] = (d[e0:e1] // NPC).astype(np.float32)
                base = ch0 + off
                idxh[c, base : base + k] = bi.reshape(k, 128)
                nmh[c, base : base + k] = bn.reshape(k, 128)
                rlh[c, base : base + k] = br.reshape(k, 128)
                off += k
            ch0 += IDX_BLK
    # device wants [128, CH]
    idxh = np.ascontiguousarray(idxh.transpose(0, 2, 1))
    nmh = np.ascontiguousarray(nmh.transpose(0, 2, 1))
    rlh = np.ascontiguousarray(rlh.transpose(0, 2, 1))
    return K, blocks, idxh, nmh, rlh


def _build(K, blocks, CH, refine_iter, Wg):
    KMAX = int(max(int(K.max()), 1))
    nc = bacc.Bacc("TRN2", target_bir_lowering=False, debug=False,
                   num_devices=NCORES)

    def din(name, shape, dt=F32):
        return nc.dram_tensor(name, shape, dt, kind="ExternalInput").ap()

    def dout(name, shape, dt=F32):
        return nc.dram_tensor(name, shape, dt, kind="ExternalOutput").ap()

    emb_d = din("emb", [BC, IN])
    idx_d = din("eidx", [128, CH], I32)
    nm_d = din("enorm", [128, CH])
    rl_d = din("erel", [128, CH])
    W1_d = din("W1", [IN, LAT])
    b1_d = din("b1c", [128, 1])
    W2_d = din("W2", [LAT, LAT])
    b2_d = din("b2c", [128, 1])
    Wbbx_d = din("Wbbx", [LAT, NN * BBX])
    bbbx_d = din("bbbxr", [128, NN * BBX])
    Wlbl_d = din("Wlbl", [LAT, NN])
    blbl_d = din("blblr", [128, NN])
    Wedge_d = din("Wedge", [LAT, NN * NN])
    bedge_d = din("bedger", [128, NN * NN])
    Wcls_d = din("Wcls", [LAT, CLS])
    bcls_d = din("bclsr", [128, CLS])
    bg_d = din("bgr", [128, BBX])
    iota_d = din("iotar", [128, 128])

    xbbx_d = dout("xbbx", [BC, NN * BBX])
    xlbl_d = dout("xlbl", [BC, NN])
    xedge_d = dout("xedge", [BC, NN * NN])
    cls_d = dout("clsp", [BC, CLS])
    ref_d = dout("refined", [BC, NN * BBX])

    mshard_d = nc.dram_tensor("mshard", [NLOC, 8], F32).ap()
    mtable_d = nc.dram_tensor("mtable", [NT, 8], F32, addr_space="Shared").ap()

    NB = 2  # batch-column halves of 512 for transposed activations
    with tile.TileContext(nc) as tc:
        # ---------------- dense phase ----------------
        with tc.tile_pool(name="wpool", bufs=1) as wp, \
             tc.tile_pool(name="dact", bufs=1) as dact, \
             tc.tile_pool(name="dwork", bufs=3) as dw, \
             tc.tile_pool(name="dpsum", bufs=4, space="PSUM") as dp:
            ident = wp.tile([128, 128], F32)
            make_identity(nc, ident[:])
            w1a = wp.tile([128, LAT], F32, tag="w1a")
            w1b = wp.tile([IN - 128, LAT], F32, tag="w1b")
            nc.sync.dma_start(out=w1a[:], in_=W1_d[0:128, :])
            nc.sync.dma_start(out=w1b[:], in_=W1_d[128:IN, :])
            b1t = wp.tile([128, 1], F32, tag="b1t")
            nc.sync.dma_start(out=b1t[:], in_=b1_d[:])
            w2t = wp.tile([128, LAT], F32, tag="w2t")
            nc.sync.dma_start(out=w2t[:], in_=W2_d[:])
            b2t = wp.tile([128, 1], F32, tag="b2t")
            nc.sync.dma_start(out=b2t[:], in_=b2_d[:])
            wbbxt = wp.tile([128, NN * BBX], F32, tag="wbbxt")
            nc.sync.dma_start(out=wbbxt[:], in_=Wbbx_d[:])
            bbbxt = wp.tile([128, NN * BBX], F32, tag="bbbxt")
            nc.sync.dma_start(out=bbbxt[:], in_=bbbx_d[:])
            wlblt = wp.tile([128, NN], F32, tag="wlblt")
            nc.sync.dma_start(out=wlblt[:], in_=Wlbl_d[:])
            blblt = wp.tile([128, NN], F32, tag="blblt")
            nc.sync.dma_start(out=blblt[:], in_=blbl_d[:])
            wedgt = wp.tile([128, NN * NN], F32, tag="wedgt")
            nc.sync.dma_start(out=wedgt[:], in_=Wedge_d[:])
            bedgt = wp.tile([128, NN * NN], F32, tag="bedgt")
            nc.sync.dma_start(out=bedgt[:], in_=bedge_d[:])
            wclst = wp.tile([128, CLS], F32, tag="wclst")
            nc.sync.dma_start(out=wclst[:], in_=Wcls_d[:])
            bclst = wp.tile([128, CLS], F32, tag="bclst")
            nc.sync.dma_start(out=bclst[:], in_=bcls_d[:])

            embT1 = dact.tile([128, BC], F32, tag="embT1")
            embT2 = dact.tile([IN - 128, BC], F32, tag="embT2")
            for r in range(BC // 128):
                et = dw.tile([128, IN], F32, tag="et")
                nc.sync.dma_start(out=et[:], in_=emb_d[r * 128:(r + 1) * 128, :])
                pt1 = dp.tile([128, 128], F32, tag="pmm")
                nc.tensor.transpose(out=pt1[:], in_=et[:, 0:128], identity=ident[:])
                nc.vector.tensor_copy(out=embT1[:, r * 128:(r + 1) * 128], in_=pt1[:])
                pt2 = dp.tile([128, 128], F32, tag="pmm")
                nc.tensor.transpose(out=pt2[: IN - 128, :], in_=et[:, 128:IN],
                                    identity=ident[:])
                nc.vector.tensor_copy(out=embT2[:, r * 128:(r + 1) * 128],
                                      in_=pt2[: IN - 128, :])

            x1T = dact.tile([128, BC], F32, tag="x1T")
            x2T = dact.tile([128, BC], F32, tag="x2T")
            x3T = dact.tile([128, BC], F32, tag="x3T")
            for h in range(NB):
                sl = slice(h * 512, (h + 1) * 512)
                p1 = dp.tile([128, 512], F32, tag="pmm")
                nc.tensor.matmul(out=p1[:], lhsT=w1a[:], rhs=embT1[:, sl],
                                 start=True, stop=False)
                nc.tensor.matmul(out=p1[:], lhsT=w1b[:], rhs=embT2[:, sl],
                                 start=False, stop=True)
                nc.scalar.activation(out=x1T[:, sl], in_=p1[:], func=AF.Sigmoid,
                                     bias=b1t[:, 0:1])
            for h in range(NB):
                sl = slice(h * 512, (h + 1) * 512)
                p2 = dp.tile([128, 512], F32, tag="pmm")
                nc.tensor.matmul(out=p2[:], lhsT=w2t[:], rhs=x1T[:, sl],
                                 start=True, stop=True)
                nc.scalar.activation(out=x2T[:, sl], in_=p2[:], func=AF.Sigmoid,
                                     bias=b2t[:, 0:1])
            for h in range(NB):
                sl = slice(h * 512, (h + 1) * 512)
                p3 = dp.tile([128, 512], F32, tag="pmm")
                nc.tensor.matmul(out=p3[:], lhsT=w2t[:], rhs=x2T[:, sl],
                                 start=True, stop=True)
                nc.scalar.activation(out=x3T[:, sl], in_=p3[:], func=AF.Sigmoid,
                                     bias=b2t[:, 0:1])

            for r in range(BC // 128):
                lhs = x3T[:, r * 128:(r + 1) * 128]
                rows = slice(r * 128, (r + 1) * 128)
                # bbx head
                pb = dp.tile([128, NN * BBX], F32, tag="pmm")
                nc.tensor.matmul(out=pb[:], lhsT=lhs, rhs=wbbxt[:],
                                 start=True, stop=True)
                sb = dw.tile([128, NN * BBX], F32, tag="sbbx")
                nc.vector.tensor_tensor(out=sb[:], in0=pb[:], in1=bbbxt[:], op=OP.add)
                nc.scalar.activation(out=sb[:], in_=sb[:], func=AF.Sigmoid)
                nc.sync.dma_start(out=xbbx_d[rows, :], in_=sb[:])
                # lbl head
                pl = dp.tile([128, NN], F32, tag="pmm")
                nc.tensor.matmul(out=pl[:], lhsT=lhs, rhs=wlblt[:],
                                 start=True, stop=True)
                slb = dw.tile([128, NN], F32, tag="slbl")
                nc.vector.tensor_tensor(out=slb[:], in0=pl[:], in1=blblt[:], op=OP.add)
                nc.scalar.activation(out=slb[:], in_=slb[:], func=AF.Sigmoid)
                nc.sync.dma_start(out=xlbl_d[rows, :], in_=slb[:])
                # edge head
                for e in range(8):
                    esl = slice(e * 512, (e + 1) * 512)
                    pe = dp.tile([128, 512], F32, tag="pmm")
                    nc.tensor.matmul(out=pe[:], lhsT=lhs, rhs=wedgt[:, esl],
                                     start=True, stop=True)
                    se = dw.tile([128, 512], F32, tag="sedge")
                    nc.vector.tensor_tensor(out=se[:], in0=pe[:], in1=bedgt[:, esl],
                                            op=OP.add)
                    nc.scalar.activation(out=se[:], in_=se[:], func=AF.Sigmoid)
                    nc.sync.dma_start(out=xedge_d[rows, esl], in_=se[:])
                # cls head
                pc = dp.tile([128, CLS], F32, tag="pmm")
                nc.tensor.matmul(out=pc[:], lhsT=lhs, rhs=wclst[:],
                                 start=True, stop=True)
                lg = dw.tile([128, CLS], F32, tag="lg")
                nc.vector.tensor_tensor(out=lg[:], in0=pc[:], in1=bclst[:], op=OP.add)
                mx = dw.tile([128, 1], F32, tag="mx")
                nc.vector.reduce_max(out=mx[:], in_=lg[:], axis=AX.X, negate=True)
                ex = dw.tile([128, CLS], F32, tag="ex")
                nc.scalar.activation(out=ex[:], in_=lg[:], func=AF.Exp,
                                     bias=mx[:, 0:1])
                sm = dw.tile([128, 1], F32, tag="sm")
                nc.vector.reduce_sum(out=sm[:], in_=ex[:], axis=AX.X)
                rc = dw.tile([128, 1], F32, tag="rc")
                nc.vector.reciprocal(out=rc[:], in_=sm[:])
                pr = dw.tile([128, CLS], F32, tag="pr")
                nc.vector.tensor_tensor(out=pr[:], in0=ex[:],
                                        in1=rc[:, 0:1].to_broadcast([128, CLS]),
                                        op=OP.mult)
                nc.sync.dma_start(out=cls_d[rows, :], in_=pr[:])

        # ---------------- GCN phase ----------------
        with tc.tile_pool(name="gconst", bufs=1) as gc, \
             tc.tile_pool(name="gbig", bufs=1) as gb, \
             tc.tile_pool(name="gblk", bufs=2) as gblk, \
             tc.tile_pool(name="gwork", bufs=3) as gw, \
             tc.tile_pool(name="gpsum", bufs=8, space="PSUM") as gp:
            iot = gc.tile([128, 128], F32, tag="iot")
            nc.sync.dma_start(out=iot[:], in_=iota_d[:])
            bgt = gc.tile([128, BBX], F32, tag="bgt")
            nc.sync.dma_start(out=bgt[:], in_=bg_d[:])

            lblt = gb.tile([128, NPC], F32, tag="lblt")
            nc.sync.dma_start(
                out=lblt[:],
                in_=xlbl_d.rearrange("(p a) d -> p (a d)", p=128))
            hcur = gb.tile([128, NPC * BBX], F32, tag="hcur")
            nc.sync.dma_start(
                out=hcur[:],
                in_=xbbx_d.rearrange("(p a) d -> p (a d)", p=128))
            h3 = hcur[:].rearrange("p (c f) -> p c f", f=BBX)
            # h0 = x_bbx * lbl
            nc.vector.tensor_tensor(
                out=h3, in0=h3,
                in1=lblt[:, :, None].to_broadcast([128, NPC, BBX]), op=OP.mult)

            mt = gb.tile([128, NPC * 8], F32, tag="mt")
            stage = gb.tile([128, NPC * BBX], F32, tag="stage")

            tile_off = np.concatenate([[0], np.cumsum(K)])
            # block-local chunk offsets for each tile
            tile_blk = {}
            for bi, blk in enumerate(blocks):
                off = 0
                for t in blk:
                    tile_blk[t] = (bi, off)
                    off += int(K[t])

            for it in range(refine_iter):
                # m = (h @ Wg), padded to 8 cols
                m3 = mt[:].rearrange("p (c f) -> p c f", f=8)
                nc.vector.memset(mt[:], 0.0)
                for jj in range(BBX):
                    acc = gw.tile([128, NPC], F32, tag="maxpy")
                    nc.vector.tensor_scalar_mul(acc[:], h3[:, :, 0],
                                                float(Wg[0, jj]))
                    tmp = gw.tile([128, NPC], F32, tag="mtmp")
                    for j in range(1, BBX):
                        nc.vector.tensor_scalar_mul(tmp[:], h3[:, :, j],
                                                    float(Wg[j, jj]))
                        nc.vector.tensor_tensor(out=acc[:], in0=acc[:], in1=tmp[:],
                                                op=OP.add)
                    nc.vector.tensor_copy(out=m3[:, :, jj], in_=acc[:])
                nc.sync.dma_start(
                    out=mshard_d.rearrange("(p a) d -> p (a d)", p=128),
                    in_=mt[:])
                nc.gpsimd.collective_compute(
                    "AllGather", OP.bypass,
                    replica_groups=[list(range(NCORES))],
                    ins=[mshard_d[:]], outs=[mtable_d[:]])

                for bi, blk in enumerate(blocks):
                    it_b = gblk.tile([128, IDX_BLK], I32, tag="itb")
                    nm_b = gblk.tile([128, IDX_BLK], F32, tag="nmb")
                    rl_b = gblk.tile([128, IDX_BLK], F32, tag="rlb")
                    csl = slice(bi * IDX_BLK, (bi + 1) * IDX_BLK)
                    nc.sync.dma_start(out=it_b[:], in_=idx_d[:, csl])
                    nc.sync.dma_start(out=nm_b[:], in_=nm_d[:, csl])
                    nc.sync.dma_start(out=rl_b[:], in_=rl_d[:, csl])
                    for t in blk:
                        k = int(K[t])
                        _, off = tile_blk[t]
                        gt = gw.tile([128, KMAX * 8], F32, tag="gt")
                        for kk in range(k):
                            nc.gpsimd.indirect_dma_start(
                                out=gt[:, kk * 8:(kk + 1) * 8],
                                out_offset=None,
                                in_=mtable_d[:],
                                in_offset=bass.IndirectOffsetOnAxis(
                                    ap=it_b[:, off + kk:off + kk + 1], axis=0),
                            )
                        gs = gw.tile([128, KMAX * 8], F32, tag="gs")
                        nc.vector.tensor_tensor(
                            out=gs[:].rearrange("p (c f) -> p c f", f=8)[:, 0:k, :],
                            in0=gt[:].rearrange("p (c f) -> p c f", f=8)[:, 0:k, :],
                            in1=nm_b[:, off:off + k, None].to_broadcast([128, k, 8]),
                            op=OP.mult)
                        sel = gw.tile([128, KMAX * 128], F32, tag="sel")
                        nc.vector.tensor_tensor(
                            out=sel[:].rearrange("p (c j) -> p c j", j=128)[:, 0:k, :],
                            in0=rl_b[:, off:off + k, None].to_broadcast([128, k, 128]),
                            in1=iot[:, None, :].to_broadcast([128, k, 128]),
                            op=OP.is_equal)
                        pt = gp.tile([128, 8], F32, tag="pt")
                        for kk in range(k):
                            nc.tensor.matmul(
                                out=pt[:],
                                lhsT=sel[:, kk * 128:(kk + 1) * 128],
                                rhs=gs[:, kk * 8:(kk + 1) * 8],
                                start=(kk == 0), stop=(kk == k - 1))
                        nc.vector.tensor_tensor(
                            out=stage[:, t * BBX:(t + 1) * BBX],
                            in0=pt[:, 0:BBX],
                            in1=bgt[:], op=OP.add)
                # corr = sigmoid(stage); refined = h + corr; h_next = refined*lbl
                nc.scalar.activation(out=stage[:], in_=stage[:], func=AF.Sigmoid)
                nc.vector.tensor_tensor(out=hcur[:], in0=hcur[:], in1=stage[:],
                                        op=OP.add)
                if it == refine_iter - 1:
                    nc.sync.dma_start(
                        out=ref_d.rearrange("(p a) d -> p (a d)", p=128),
                        in_=hcur[:])
                else:
                    nc.vector.tensor_tensor(
                        out=h3, in0=h3,
                        in1=lblt[:, :, None].to_broadcast([128, NPC, BBX]),
                        op=OP.mult)

    nc.compile()
    return nc


def kernel(embedding, E, refine_iter, W1, b1, W2, b2, Wbbx, bbbx,
           Wlbl, blbl, Wedge, bedge, Wcls, bcls, Wg, bg):
    refine_iter = int(refine_iter)
    embedding = np.asarray(embedding, np.float32)
    E = np.asarray(E)
    K, blocks, idxh, nmh, rlh = _prep_edges(E)
    CH = len(blocks) * IDX_BLK
    nc = _build(K, blocks, CH, refine_iter, np.asarray(Wg, np.float64))

    rep = lambda v: np.ascontiguousarray(
        np.broadcast_to(np.asarray(v, np.float32)[None, :], (128, len(v))))
    iota = np.broadcast_to(np.arange(128, dtype=np.float32)[None, :], (128, 128))
    common = {
        "W1": np.asarray(W1, np.float32), "b1c": np.asarray(b1, np.float32).reshape(128, 1),
        "W2": np.asarray(W2, np.float32), "b2c": np.asarray(b2, np.float32).reshape(128, 1),
        "Wbbx": np.asarray(Wbbx, np.float32), "bbbxr": rep(np.asarray(bbbx, np.float32)),
        "Wlbl": np.asarray(Wlbl, np.float32), "blblr": rep(np.asarray(blbl, np.float32)),
        "Wedge": np.asarray(Wedge, np.float32), "bedger": rep(np.asarray(bedge, np.float32)),
        "Wcls": np.asarray(Wcls, np.float32), "bclsr": rep(np.asarray(bcls, np.float32)),
        "Wg": np.asarray(Wg, np.float32), "bgr": rep(np.asarray(bg, np.float32)),
        "iotar": np.ascontiguousarray(iota),
    }
    in_maps = []
    for c in range(NCORES):
        im = dict(common)
        im["emb"] = np.ascontiguousarray(embedding[c * BC:(c + 1) * BC])
        im["eidx"] = idxh[c]
        im["enorm"] = nmh[c]
        im["erel"] = rlh[c]
        in_maps.append(im)

    res = run_bass_kernel_spmd(nc, in_maps, core_ids=list(range(NCORES)),
                              trace=bool(PROFILE))
    global LAST_EXEC_NS
    LAST_EXEC_NS = res.exec_time_ns
    if res.exec_time_ns is not None:
        print(f"HW exec time: {res.exec_time_ns} ns", flush=True)
    outs = res.results
    cat = lambda k: np.concatenate([outs[c][k] for c in range(NCORES)], axis=0)
    x_bbx = cat("xbbx").reshape(B, NN, BBX)
    x_lbl = cat("xlbl").reshape(B, NN, 1)
    x_edge = cat("xedge").reshape(B, NN, NN)
    class_pred = cat("clsp").reshape(B, CLS)
    if refine_iter == 0:
        refined = x_bbx.copy()
    else:
        refined = cat("refined").reshape(B, NN, BBX)
    return (x_bbx, x_lbl, x_edge, class_pred, refined)
